# revision 41
# baseline (speedup 1.0000x reference)
"""Trainium2 Bass kernel for DistanceNeighborList (min-image pairwise distances).

Contract: kernel(x, cell, pbc) takes FULL inputs (x [4,2048,3] f32,
cell [3,3] f32, pbc [3] bool) and returns (dist [4,2048,2048] f32,
mask [4,2048,2048] bool) matching reference.py.

Sharding — cyclic-star symmetric decomposition:
  dist[b] is symmetric.  View it as a 16x16 grid of 128x128 blocks and
  orient K16 as a circulant tournament: block x "owns" partner blocks
  {x..x+8} (mod 16, 9 blocks) for x in 0..7 and {x..x+7} (8 blocks)
  for x in 8..15.  Every unordered block pair is owned exactly once.
  A slot computes one star: rows = center block, cols = its window
  (cyclically contiguous), then mirrors each 128x128 block via PE
  transpose to the transposed position.  All 8 cores run the same
  8-slot program (centers at device positions 0,2,..,14; widths
  9,9,9,9,8,8,8,8 blocks); core h of batch b feeds inputs rotated by
  h*128 atoms, so the two cores of a batch cover the odd/even real
  stars.  Host un-rotates with np.roll and sums the two disjoint
  outputs (PJRT zero-fills output buffers).  mask = dist > 0 exactly
  reproduces the reference mask.

Math (per pair, all f32):
  frac = x @ inv(cell) per atom (host)
  w_k = wrap(frac_ik - frac_jk) into [-0.5, 0.5]   (custom DVE
        ADD_RANGE_WRAP) — exactly 0 on the diagonal.
  d2  = sum_c (L_cc * u_c)^2 where u built from w via Cholesky L of
        G = cell @ cell.T (scales folded into ACT Square).
  dist = sqrt(d2) * (d2 < 25)   [sqrt(0) = 0 covers the d2 > 0 term]
"""

import numpy as np
from contextlib import ExitStack

import concourse.bass as bass
import concourse.bacc as bacc
import concourse.mybir as mybir
from concourse import tile
from concourse.bass_utils import run_bass_kernel_spmd

F32 = np.float32
B, N = 4, 2048
NCORES = 8
RB = 128                       # block edge
NBLK = N // RB                 # 16
CUT2 = 25.0

CENTERS = (0, 2, 4, 6, 8, 10, 12, 14)       # device block positions
WIDTHS = (9, 9, 9, 9, 8, 8, 8, 8)           # window size in blocks
BCW = 22 * RB                               # bc extent: max window end

AO = mybir.AluOpType
AF = mybir.ActivationFunctionType
DT = mybir.dt


def _host_prep(x, cell):
    """Per-atom fractional coords and Cholesky constants, all f32."""
    x = np.ascontiguousarray(x, dtype=F32)
    cell_f = np.ascontiguousarray(cell, dtype=F32)
    inv_cell = np.linalg.inv(cell_f).astype(F32)
    frac = (x @ inv_cell).astype(F32)                  # [B, N, 3]
    nfx = (-frac).astype(F32)

    G = cell_f.astype(np.float64) @ cell_f.astype(np.float64).T
    L = np.linalg.cholesky(G)
    L00, L11, L22 = F32(L[0, 0]), F32(L[1, 1]), F32(L[2, 2])
    consts = dict(
        L00=L00, L11=L11, L22=L22,
        lam10=F32(F32(L[1, 0]) / L00),
        lam20=F32(F32(L[2, 0]) / L00),
        lam21=F32(F32(L[2, 1]) / L11),
    )
    return frac, nfx, consts


def _build_program(c):
    """One SPMD Bass program: 8 star slots + transposed mirrors."""
    nc = bacc.Bacc("TRN2", debug=False, num_devices=NCORES)

    # s_arr[p, si*3+k] = frac of the slot-si center-block atom p
    s_in = nc.dram_tensor("s_arr", [RB, 8 * 3], DT.float32, kind="ExternalInput")
    # bc[p, k, t] = -frac of device column t (rotated per core, doubled)
    bc_in = nc.dram_tensor("bc", [RB, 3, BCW], DT.float32, kind="ExternalInput")
    id_in = nc.dram_tensor("ident", [RB, RB], DT.float32, kind="ExternalInput")
    dist_out = nc.dram_tensor("dist", [N, N], DT.float32, kind="ExternalOutput")

    with tile.TileContext(nc) as tc, ExitStack() as ctx:
        cpool = ctx.enter_context(tc.tile_pool(name="const", bufs=1))
        s_sb = cpool.tile([RB, 8 * 3], DT.float32)
        nc.sync.dma_start(s_sb[:], s_in[:])
        # per-component tiles, loaded in window-sized pieces so each slot's
        # columns land just in time (slot 0 needs only [0, 9) blocks); the
        # first pieces go out on three different engine queues in parallel
        bc_ks = []
        for k in range(3):
            bck = cpool.tile([RB, BCW], DT.float32, tag=f"bc{k}")
            bc_ks.append(bck)
        first_eng = (nc.sync, nc.gpsimd, nc.scalar)
        for k in range(3):
            first_eng[k].dma_start(bc_ks[k][:, 0:9 * RB], bc_in[:, k, 0:9 * RB])
        for lo, hi in ((9 * RB, 16 * RB), (16 * RB, BCW)):
            for k in range(3):
                nc.sync.dma_start(bc_ks[k][:, lo:hi], bc_in[:, k, lo:hi])
        id_sb = cpool.tile([RB, RB], DT.float32)
        nc.sync.dma_start(id_sb[:], id_in[:])

        wpool = ctx.enter_context(tc.tile_pool(name="work", bufs=4))
        opool = ctx.enter_context(tc.tile_pool(name="out", bufs=4))
        ppool = ctx.enter_context(tc.tile_pool(name="mirror", bufs=2, space="PSUM"))

        # 3-stage software pipeline over work items with skew: stage A of
        # item i is emitted before stage B of item i-1 and stage C of item
        # i-2, so the in-order DVE queue has wrap work to chew on while
        # ACT finishes the previous items' squares / sqrt.  The last slot
        # is split into two half-windows so the pipeline drain overlaps.
        # item = (si, c0, nb): slot si, window blocks [P+c0, P+c0+nb)
        ITEMS = [(si, 0, WIDTHS[si]) for si in range(7)] + [(7, 0, 4), (7, 4, 4)]
        slot = {}

        def stage_a(item):
            si, c0, nb = item
            P = CENTERS[si]
            W = nb * RB
            O = (P + c0) * RB
            w0 = wpool.tile([RB, W], DT.float32, tag="w0")
            w1 = wpool.tile([RB, W], DT.float32, tag="w1")
            w2 = wpool.tile([RB, W], DT.float32, tag="w2")
            for k, wk in enumerate((w0, w1, w2)):
                nc.vector.add_range_wrap(
                    wk[:], bc_ks[k][:, O:O + W],
                    s_sb[:, si * 3 + k:si * 3 + k + 1], 0.5, 1.0)
            u1 = wpool.tile([RB, W], DT.float32, tag="u1")
            nc.vector.scalar_tensor_tensor(
                u1[:], w2[:], float(c["lam21"]), w1[:], AO.mult, AO.add)
            t0 = wpool.tile([RB, W], DT.float32, tag="t0")
            nc.vector.scalar_tensor_tensor(
                t0[:], w1[:], float(c["lam10"]), w0[:], AO.mult, AO.add)
            if float(c["lam20"]) != 0.0:
                # zero for near-tridiagonal Gram matrices (a0.a2 == 0):
                # t0 + 0*w2 == t0 bit-exactly, so the op can be skipped
                nc.vector.scalar_tensor_tensor(
                    t0[:], w2[:], float(c["lam20"]), t0[:], AO.mult, AO.add)
            nc.scalar.activation(t0[:], t0[:], AF.Square, scale=float(c["L00"]))
            nc.scalar.activation(u1[:], u1[:], AF.Square, scale=float(c["L11"]))
            nc.scalar.activation(w2[:], w2[:], AF.Square, scale=float(c["L22"]))
            slot[item] = (w0, u1, t0, w2)

        def stage_b(item):
            w0, u1, t0, w2 = slot[item]
            nc.vector.tensor_add(t0[:], t0[:], u1[:])
            nc.vector.tensor_add(t0[:], t0[:], w2[:])     # d2 in t0
            nc.scalar.activation(w0[:], t0[:], AF.Sqrt)   # r in w0

        def stage_c(item):
            w0, u1, t0, w2 = slot.pop(item)
            si, c0, nb = item
            P = CENTERS[si]
            W = nb * RB
            O = (P + c0) * RB              # window device column start
            OC = P * RB                    # center block (rows of direct)
            dist_t = opool.tile([RB, W], DT.float32, tag="dist")
            # dist = (d2 < 25) * sqrt(d2), in two column halves so the
            # mirror transposes can start on the first half while the
            # vector engine finishes the second
            H = (nb // 2) * RB if nb > 1 else W
            nc.vector.scalar_tensor_tensor(
                dist_t[:, 0:H], t0[:, 0:H], CUT2, w0[:, 0:H],
                AO.is_lt, AO.mult)
            if H < W:
                nc.vector.scalar_tensor_tensor(
                    dist_t[:, H:W], t0[:, H:W], CUT2, w0[:, H:W],
                    AO.is_lt, AO.mult)

            # direct part: rows = center block, cols = window folded at N
            if O >= N:
                nc.sync.dma_start(
                    dist_out[OC:OC + RB, O - N:O - N + W], dist_t[:])
            else:
                w1_cols = min(W, N - O)
                nc.sync.dma_start(
                    dist_out[OC:OC + RB, O:O + w1_cols], dist_t[:, 0:w1_cols])
                if w1_cols < W:
                    nc.sync.dma_start(
                        dist_out[OC:OC + RB, 0:W - w1_cols],
                        dist_t[:, w1_cols:W])

            # mirror: transpose every window block into PSUM, then write
            # the transposed block-column (window rows x center cols)
            tp = ppool.tile([RB, W], DT.float32, tag="tp")
            for ci in range(nb):
                nc.tensor.transpose(
                    tp[:, ci * RB:(ci + 1) * RB],
                    dist_t[:, ci * RB:(ci + 1) * RB],
                    id_sb[:])
            # DMA cannot read PSUM — bounce through SBUF on the scalar engine
            mir = opool.tile([RB, W], DT.float32, tag="mir")
            nc.scalar.activation(mir[:], tp[:], AF.Copy)
            # mirror rows are window blocks [P+c0, P+c0+nb) mod NBLK;
            # split at the wrap into contiguous runs
            b0 = (P + c0) % NBLK
            mb1 = min(nb, NBLK - b0)
            nc.gpsimd.dma_start(
                dist_out[b0 * RB:(b0 + mb1) * RB, OC:OC + RB].rearrange(
                    "(ci p) q -> p ci q", p=RB),
                mir[:, 0:mb1 * RB].rearrange("p (ci q) -> p ci q", q=RB))
            if mb1 < nb:
                mb2 = nb - mb1
                nc.gpsimd.dma_start(
                    dist_out[0:mb2 * RB, OC:OC + RB].rearrange(
                        "(ci p) q -> p ci q", p=RB),
                    mir[:, mb1 * RB:W].rearrange("p (ci q) -> p ci q", q=RB))

        for i in range(len(ITEMS) + 2):
            if i < len(ITEMS):
                stage_a(ITEMS[i])
            if 1 <= i <= len(ITEMS):
                stage_b(ITEMS[i - 1])
            if i >= 2:
                stage_c(ITEMS[i - 2])
    nc.compile()
    return nc


def _prepare(x, cell):
    """Host prep -> (in_maps for the 8 cores, cholesky consts)."""
    frac, nfx, consts = _host_prep(x, cell)
    ident = np.eye(RB, dtype=F32)
    in_maps = []
    for core in range(NCORES):
        b, h = divmod(core, 2)
        rolled_frac = np.roll(frac[b], -RB * h, axis=0)
        rolled_nfx = np.roll(nfx[b], -RB * h, axis=0)
        s_arr = np.empty((RB, 8 * 3), F32)
        for si, P in enumerate(CENTERS):
            s_arr[:, si * 3:si * 3 + 3] = rolled_frac[P * RB:(P + 1) * RB, :]
        ext = np.concatenate([rolled_nfx, rolled_nfx[:BCW - N]], axis=0)  # [BCW,3]
        bc = np.ascontiguousarray(
            np.broadcast_to(ext.T[None, :, :], (RB, 3, BCW)))
        in_maps.append({"s_arr": s_arr, "bc": bc, "ident": ident})
    return in_maps, consts


def _gather(results):
    dist = np.empty((B, N, N), F32)
    for b in range(B):
        dist[b] = results[2 * b]["dist"] + np.roll(
            results[2 * b + 1]["dist"], RB, axis=(0, 1))
    return dist, dist > 0


def _reference_fallback(x, cell, pbc):
    """Numpy replica of reference.py for non-standard pbc/shapes."""
    x = np.asarray(x, F32)
    cell = np.asarray(cell, F32)
    inv_cell = np.linalg.inv(cell).astype(F32)
    out_d = np.empty((x.shape[0], x.shape[1], x.shape[1]), F32)
    out_m = np.empty(out_d.shape, bool)
    pbcf = np.asarray(pbc).astype(F32)
    for b in range(x.shape[0]):
        diff = x[b][:, None, :] - x[b][None, :, :]
        frac = (diff @ inv_cell).astype(F32)
        frac = (frac - np.round(frac) * pbcf).astype(F32)
        diff = (frac @ cell).astype(F32)
        d2 = np.einsum("ijk,ijk->ij", diff, diff).astype(F32)
        m = (d2 < F32(CUT2)) & (d2 > 0)
        out_d[b] = np.sqrt(np.where(m, d2, F32(1.0))).astype(F32) * m
        out_m[b] = m
    return out_d, out_m


def kernel(x, cell, pbc):
    x = np.asarray(x)
    pbc_arr = np.asarray(pbc)
    if x.shape != (B, N, 3) or not bool(pbc_arr.all()):
        return _reference_fallback(x, cell, pbc)

    in_maps, consts = _prepare(x, cell)
    nc = _build_program(consts)
    res = run_bass_kernel_spmd(nc, in_maps, list(range(NCORES)))
    return _gather(res.results)


# revision 43
# speedup vs baseline: 1.0139x; 1.0139x over previous
"""Trainium2 Bass kernel for DistanceNeighborList (min-image pairwise distances).

Contract: kernel(x, cell, pbc) takes FULL inputs (x [4,2048,3] f32,
cell [3,3] f32, pbc [3] bool) and returns (dist [4,2048,2048] f32,
mask [4,2048,2048] bool) matching reference.py.

Sharding — cyclic-star symmetric decomposition:
  dist[b] is symmetric.  View it as a 16x16 grid of 128x128 blocks and
  orient K16 as a circulant tournament: block x "owns" partner blocks
  {x..x+8} (mod 16, 9 blocks) for x in 0..7 and {x..x+7} (8 blocks)
  for x in 8..15.  Every unordered block pair is owned exactly once.
  A slot computes one star: rows = center block, cols = its window
  (cyclically contiguous), then mirrors each 128x128 block via PE
  transpose to the transposed position.  All 8 cores run the same
  8-slot program (centers at device positions 0,2,..,14; widths
  9,9,9,9,8,8,8,8 blocks); core h of batch b feeds inputs rotated by
  h*128 atoms, so the two cores of a batch cover the odd/even real
  stars.  Host un-rotates with np.roll and sums the two disjoint
  outputs (PJRT zero-fills output buffers).  mask = dist > 0 exactly
  reproduces the reference mask.

Math (per pair, all f32):
  frac = x @ inv(cell) per atom (host)
  w_k = wrap(frac_ik - frac_jk) into [-0.5, 0.5]   (custom DVE
        ADD_RANGE_WRAP) — exactly 0 on the diagonal.
  d2  = sum_c (L_cc * u_c)^2 where u built from w via Cholesky L of
        G = cell @ cell.T (scales folded into ACT Square).
  dist = sqrt(d2) * (d2 < 25)   [sqrt(0) = 0 covers the d2 > 0 term]
"""

import numpy as np
from contextlib import ExitStack

import concourse.bass as bass
import concourse.bacc as bacc
import concourse.mybir as mybir
from concourse import tile
from concourse.bass_utils import run_bass_kernel_spmd

F32 = np.float32
B, N = 4, 2048
NCORES = 8
RB = 128                       # block edge
NBLK = N // RB                 # 16
CUT2 = 25.0

CENTERS = (0, 2, 4, 6, 8, 10, 12, 14)       # device block positions
WIDTHS = (9, 9, 9, 9, 8, 8, 8, 8)           # window size in blocks
BCW = 22 * RB                               # bc extent: max window end

AO = mybir.AluOpType
AF = mybir.ActivationFunctionType
DT = mybir.dt


def _host_prep(x, cell):
    """Per-atom fractional coords and Cholesky constants, all f32."""
    x = np.ascontiguousarray(x, dtype=F32)
    cell_f = np.ascontiguousarray(cell, dtype=F32)
    inv_cell = np.linalg.inv(cell_f).astype(F32)
    frac = (x @ inv_cell).astype(F32)                  # [B, N, 3]
    nfx = (-frac).astype(F32)

    G = cell_f.astype(np.float64) @ cell_f.astype(np.float64).T
    L = np.linalg.cholesky(G)
    L00, L11, L22 = F32(L[0, 0]), F32(L[1, 1]), F32(L[2, 2])
    consts = dict(
        L00=L00, L11=L11, L22=L22,
        lam10=F32(F32(L[1, 0]) / L00),
        lam20=F32(F32(L[2, 0]) / L00),
        lam21=F32(F32(L[2, 1]) / L11),
    )
    return frac, nfx, consts


def _build_program(c):
    """One SPMD Bass program: 8 star slots + transposed mirrors."""
    nc = bacc.Bacc("TRN2", debug=False, num_devices=NCORES)

    # s_arr[p, si*3+k] = frac of the slot-si center-block atom p
    s_in = nc.dram_tensor("s_arr", [RB, 8 * 3], DT.float32, kind="ExternalInput")
    # bc[p, k, t] = -frac of device column t (rotated per core, doubled)
    bc_in = nc.dram_tensor("bc", [RB, 3, BCW], DT.float32, kind="ExternalInput")
    id_in = nc.dram_tensor("ident", [RB, RB], DT.float32, kind="ExternalInput")
    dist_out = nc.dram_tensor("dist", [N, N], DT.float32, kind="ExternalOutput")

    with tile.TileContext(nc) as tc, ExitStack() as ctx:
        cpool = ctx.enter_context(tc.tile_pool(name="const", bufs=1))
        s_sb = cpool.tile([RB, 8 * 3], DT.float32)
        nc.sync.dma_start(s_sb[:], s_in[:])
        # per-component tiles, loaded in window-sized pieces so each slot's
        # columns land just in time (slot 0 needs only [0, 9) blocks); the
        # first pieces go out on three different engine queues in parallel
        bc_ks = []
        for k in range(3):
            bck = cpool.tile([RB, BCW], DT.float32, tag=f"bc{k}")
            bc_ks.append(bck)
        first_eng = (nc.sync, nc.gpsimd, nc.scalar)
        for k in range(3):
            first_eng[k].dma_start(bc_ks[k][:, 0:9 * RB], bc_in[:, k, 0:9 * RB])
        for lo, hi in ((9 * RB, 16 * RB), (16 * RB, BCW)):
            for k in range(3):
                nc.sync.dma_start(bc_ks[k][:, lo:hi], bc_in[:, k, lo:hi])
        id_sb = cpool.tile([RB, RB], DT.float32)
        nc.sync.dma_start(id_sb[:], id_in[:])

        wpool = ctx.enter_context(tc.tile_pool(name="work", bufs=4))
        opool = ctx.enter_context(tc.tile_pool(name="out", bufs=4))
        ppool = ctx.enter_context(tc.tile_pool(name="mirror", bufs=2, space="PSUM"))

        # 3-stage software pipeline over work items with skew: stage A of
        # item i is emitted before stage B of item i-1 and stage C of item
        # i-2, so the in-order DVE queue has wrap work to chew on while
        # ACT finishes the previous items' squares / sqrt.  The last slot
        # is split into two half-windows so the pipeline drain overlaps.
        # item = (si, c0, nb): slot si, window blocks [P+c0, P+c0+nb)
        ITEMS = [(si, 0, WIDTHS[si]) for si in range(7)] + [(7, 0, 4), (7, 4, 4)]
        slot = {}

        def stage_a(item):
            si, c0, nb = item
            P = CENTERS[si]
            W = nb * RB
            O = (P + c0) * RB
            w0 = wpool.tile([RB, W], DT.float32, tag="w0")
            w1 = wpool.tile([RB, W], DT.float32, tag="w1")
            w2 = wpool.tile([RB, W], DT.float32, tag="w2")
            for k, wk in enumerate((w0, w1, w2)):
                nc.vector.add_range_wrap(
                    wk[:], bc_ks[k][:, O:O + W],
                    s_sb[:, si * 3 + k:si * 3 + k + 1], 0.5, 1.0)
            u1 = wpool.tile([RB, W], DT.float32, tag="u1")
            nc.vector.scalar_tensor_tensor(
                u1[:], w2[:], float(c["lam21"]), w1[:], AO.mult, AO.add)
            t0 = wpool.tile([RB, W], DT.float32, tag="t0")
            nc.vector.scalar_tensor_tensor(
                t0[:], w1[:], float(c["lam10"]), w0[:], AO.mult, AO.add)
            if float(c["lam20"]) != 0.0:
                # zero for near-tridiagonal Gram matrices (a0.a2 == 0):
                # t0 + 0*w2 == t0 bit-exactly, so the op can be skipped
                nc.vector.scalar_tensor_tensor(
                    t0[:], w2[:], float(c["lam20"]), t0[:], AO.mult, AO.add)
            nc.scalar.activation(t0[:], t0[:], AF.Square, scale=float(c["L00"]))
            nc.scalar.activation(u1[:], u1[:], AF.Square, scale=float(c["L11"]))
            nc.scalar.activation(w2[:], w2[:], AF.Square, scale=float(c["L22"]))
            slot[item] = (w0, u1, t0, w2)

        def stage_b(item):
            w0, u1, t0, w2 = slot[item]
            nc.vector.tensor_add(t0[:], t0[:], u1[:])
            nc.vector.tensor_add(t0[:], t0[:], w2[:])     # d2 in t0
            nc.scalar.activation(w0[:], t0[:], AF.Sqrt)   # r in w0

        def stage_c(item):
            w0, u1, t0, w2 = slot.pop(item)
            si, c0, nb = item
            P = CENTERS[si]
            W = nb * RB
            O = (P + c0) * RB              # window device column start
            OC = P * RB                    # center block (rows of direct)
            dist_t = opool.tile([RB, W], DT.float32, tag="dist")
            # dist = (d2 < 25) * sqrt(d2) in one fused op
            nc.vector.scalar_tensor_tensor(
                dist_t[:], t0[:], CUT2, w0[:], AO.is_lt, AO.mult)

            # direct part: rows = center block, cols = window folded at N
            if O >= N:
                nc.sync.dma_start(
                    dist_out[OC:OC + RB, O - N:O - N + W], dist_t[:])
            else:
                w1_cols = min(W, N - O)
                nc.sync.dma_start(
                    dist_out[OC:OC + RB, O:O + w1_cols], dist_t[:, 0:w1_cols])
                if w1_cols < W:
                    nc.sync.dma_start(
                        dist_out[OC:OC + RB, 0:W - w1_cols],
                        dist_t[:, w1_cols:W])

            # mirror: transpose window blocks into PSUM, then write the
            # transposed block-column (window rows x center cols).  For
            # c0 == 0 the first window block is the diagonal block, whose
            # transpose equals what the direct DMA already wrote — skip it.
            cs = RB if c0 == 0 else 0
            tp = ppool.tile([RB, W], DT.float32, tag="tp")
            for ci in range(cs // RB, nb):
                nc.tensor.transpose(
                    tp[:, ci * RB:(ci + 1) * RB],
                    dist_t[:, ci * RB:(ci + 1) * RB],
                    id_sb[:])
            # DMA cannot read PSUM — bounce through SBUF on the scalar engine
            mir = opool.tile([RB, W], DT.float32, tag="mir")
            nc.scalar.activation(mir[:, cs:W], tp[:, cs:W], AF.Copy)
            # mirror rows are window blocks [P+c0+cs/RB, P+c0+nb) mod NBLK;
            # split at the wrap into contiguous runs
            nbm = nb - cs // RB
            b0 = (P + c0 + cs // RB) % NBLK
            mb1 = min(nbm, NBLK - b0)
            nc.gpsimd.dma_start(
                dist_out[b0 * RB:(b0 + mb1) * RB, OC:OC + RB].rearrange(
                    "(ci p) q -> p ci q", p=RB),
                mir[:, cs:cs + mb1 * RB].rearrange("p (ci q) -> p ci q", q=RB))
            if mb1 < nbm:
                mb2 = nbm - mb1
                nc.gpsimd.dma_start(
                    dist_out[0:mb2 * RB, OC:OC + RB].rearrange(
                        "(ci p) q -> p ci q", p=RB),
                    mir[:, cs + mb1 * RB:W].rearrange("p (ci q) -> p ci q", q=RB))

        for i in range(len(ITEMS) + 2):
            if i < len(ITEMS):
                stage_a(ITEMS[i])
            if 1 <= i <= len(ITEMS):
                stage_b(ITEMS[i - 1])
            if i >= 2:
                stage_c(ITEMS[i - 2])
    nc.compile()
    return nc


def _prepare(x, cell):
    """Host prep -> (in_maps for the 8 cores, cholesky consts)."""
    frac, nfx, consts = _host_prep(x, cell)
    ident = np.eye(RB, dtype=F32)
    in_maps = []
    for core in range(NCORES):
        b, h = divmod(core, 2)
        rolled_frac = np.roll(frac[b], -RB * h, axis=0)
        rolled_nfx = np.roll(nfx[b], -RB * h, axis=0)
        s_arr = np.empty((RB, 8 * 3), F32)
        for si, P in enumerate(CENTERS):
            s_arr[:, si * 3:si * 3 + 3] = rolled_frac[P * RB:(P + 1) * RB, :]
        ext = np.concatenate([rolled_nfx, rolled_nfx[:BCW - N]], axis=0)  # [BCW,3]
        bc = np.ascontiguousarray(
            np.broadcast_to(ext.T[None, :, :], (RB, 3, BCW)))
        in_maps.append({"s_arr": s_arr, "bc": bc, "ident": ident})
    return in_maps, consts


def _gather(results):
    dist = np.empty((B, N, N), F32)
    for b in range(B):
        dist[b] = results[2 * b]["dist"] + np.roll(
            results[2 * b + 1]["dist"], RB, axis=(0, 1))
    return dist, dist > 0


def _reference_fallback(x, cell, pbc):
    """Numpy replica of reference.py for non-standard pbc/shapes."""
    x = np.asarray(x, F32)
    cell = np.asarray(cell, F32)
    inv_cell = np.linalg.inv(cell).astype(F32)
    out_d = np.empty((x.shape[0], x.shape[1], x.shape[1]), F32)
    out_m = np.empty(out_d.shape, bool)
    pbcf = np.asarray(pbc).astype(F32)
    for b in range(x.shape[0]):
        diff = x[b][:, None, :] - x[b][None, :, :]
        frac = (diff @ inv_cell).astype(F32)
        frac = (frac - np.round(frac) * pbcf).astype(F32)
        diff = (frac @ cell).astype(F32)
        d2 = np.einsum("ijk,ijk->ij", diff, diff).astype(F32)
        m = (d2 < F32(CUT2)) & (d2 > 0)
        out_d[b] = np.sqrt(np.where(m, d2, F32(1.0))).astype(F32) * m
        out_m[b] = m
    return out_d, out_m


def kernel(x, cell, pbc):
    x = np.asarray(x)
    pbc_arr = np.asarray(pbc)
    if x.shape != (B, N, 3) or not bool(pbc_arr.all()):
        return _reference_fallback(x, cell, pbc)

    in_maps, consts = _prepare(x, cell)
    nc = _build_program(consts)
    res = run_bass_kernel_spmd(nc, in_maps, list(range(NCORES)))
    return _gather(res.results)
